# revision 12
# baseline (speedup 1.0000x reference)
"""CVNetRerank Trainium2 kernel.

Data-parallel over the 16 candidate pairs: 8 NeuronCores x 2 images each.
Device computes (per core): 3-scale bilinear resize + 3x3 conv (1024->256,
float32r matmuls), channel L2-norm, the 9 cross-scale correlations, the
separable 4D bilinear resize of each correlation to 16^4 and the ReLU.
Host computes the tiny CVLearner tail (~2% of FLOPs) in numpy.
"""

import numpy as np
from contextlib import ExitStack

N_CORES = 8
B, C, H, W = 16, 1024, 16, 16
RED = 256
IMG_PER_CORE = B // N_CORES
SCALE_HW = [4, 8, 16]  # spatial size per scale index (0.25, 0.5, 1.0)
EPS = 1e-5

LAYERS = [(9, 16, 5, 2), (16, 16, 3, 1), (16, 32, 3, 2),
          (32, 32, 3, 1), (32, 32, 3, 1), (32, 64, 3, 2)]


# ----------------------------------------------------------------------------
# host-side helpers
# ----------------------------------------------------------------------------

def _resize_matrix(src, dst):
    """Bilinear align_corners 1D resize matrix [dst, src]."""
    R = np.zeros((dst, src), np.float32)
    if dst == 1:
        R[0, 0] = 1.0
        return R
    ys = np.linspace(0.0, src - 1.0, dst)
    y0 = np.clip(np.floor(ys).astype(np.int64), 0, src - 1)
    y1 = np.minimum(y0 + 1, src - 1)
    wy = (ys - y0).astype(np.float32)
    for j in range(dst):
        R[j, y0[j]] += 1.0 - wy[j]
        R[j, y1[j]] += wy[j]
    return R


def _half_resize_weights():
    """Per-output-row/col lerp weights for the 16->8 resize on an affine grid.

    Using grid rows y0=0,2,..,14 / y1=y0+1; row 7 maps to (w0,w1)=(0,1) so it
    picks source row 15 exactly (matches align_corners endpoints)."""
    ys = np.linspace(0.0, 15.0, 8)
    y0 = np.floor(ys).astype(np.int64)
    w1 = (ys - y0).astype(np.float32)
    w0 = 1.0 - w1
    # replace last row with grid (14,15) weights (0,1)
    w0[7], w1[7] = 0.0, 1.0
    return w0, w1


def _build_consts(params):
    """Pack weights/constants for the device kernel. Returns dict name->np."""
    c = {}
    # conv weights: [3, 9, 1024, 256] as lhsT per (scale, tap)
    wc = np.empty((3, 9, C, RED), np.float32)
    for s in range(3):
        w = np.asarray(params['conv2d_w%d' % s], np.float32)  # [256,1024,3,3]
        for dy in range(3):
            for dx in range(3):
                wc[s, dy * 3 + dx] = w[:, :, dy, dx].T
    import ml_dtypes
    c['wconv'] = wc.astype(ml_dtypes.bfloat16)
    # 4D-resize matrices, transposed for lhsT: [src*src, 256]
    for s, hw in enumerate(SCALE_HW[:2]):
        R1 = _resize_matrix(hw, 16)
        R2 = np.kron(R1, R1).astype(np.float32)  # [256, hw*hw]
        import ml_dtypes
        c['rmat%d' % s] = np.ascontiguousarray(R2.T).astype(ml_dtypes.bfloat16)
    # half-resize DVE weight tiles, replicated over 128 partitions and 8 chunks
    w0, w1 = _half_resize_weights()
    wh0 = np.broadcast_to(w0[None, None, :, None], (128, 8, 8, 16))
    wh1 = np.broadcast_to(w1[None, None, :, None], (128, 8, 8, 16))
    ww0 = np.broadcast_to(w0[None, None, None, :], (128, 8, 8, 8))
    ww1 = np.broadcast_to(w1[None, None, None, :], (128, 8, 8, 8))
    c['wh0'] = np.ascontiguousarray(wh0, np.float32)
    c['wh1'] = np.ascontiguousarray(wh1, np.float32)
    c['ww0'] = np.ascontiguousarray(ww0, np.float32)
    c['ww1'] = np.ascontiguousarray(ww1, np.float32)
    return c


# ----------------------------------------------------------------------------
# device program
# ----------------------------------------------------------------------------

def _build_program():
    import concourse.bacc as bacc
    import concourse.bass as bass
    import concourse.tile as tile
    import concourse.mybir as mybir
    import concourse.bass_isa as bass_isa
    from concourse.masks import make_identity

    f32 = mybir.dt.float32
    bf16 = mybir.dt.bfloat16

    nc = bacc.Bacc("TRN2", target_bir_lowering=False, debug=False)

    qf = nc.dram_tensor("qf", [IMG_PER_CORE, C, H, W], f32, kind="ExternalInput").ap()
    kf = nc.dram_tensor("kf", [IMG_PER_CORE, C, H, W], f32, kind="ExternalInput").ap()
    wconv = nc.dram_tensor("wconv", [3, 9, C, RED], bf16, kind="ExternalInput").ap()
    rmat0 = nc.dram_tensor("rmat0", [16, 256], bf16, kind="ExternalInput").ap()
    rmat1 = nc.dram_tensor("rmat1", [64, 256], bf16, kind="ExternalInput").ap()
    wh0 = nc.dram_tensor("wh0", [128, 8, 8, 16], f32, kind="ExternalInput").ap()
    wh1 = nc.dram_tensor("wh1", [128, 8, 8, 16], f32, kind="ExternalInput").ap()
    ww0 = nc.dram_tensor("ww0", [128, 8, 8, 8], f32, kind="ExternalInput").ap()
    ww1 = nc.dram_tensor("ww1", [128, 8, 8, 8], f32, kind="ExternalInput").ap()
    xout = nc.dram_tensor("xout", [IMG_PER_CORE, 9, 256, 256], f32,
                          kind="ExternalOutput").ap()

    Relu = mybir.ActivationFunctionType.Relu
    Sqrt = mybir.ActivationFunctionType.Sqrt

    with tile.TileContext(nc) as tc, ExitStack() as ctx:
        consts = ctx.enter_context(tc.tile_pool(name="consts", bufs=1))
        inpool = ctx.enter_context(tc.tile_pool(name="inp", bufs=1))
        wpool = ctx.enter_context(tc.tile_pool(name="wts", bufs=6))
        fpool = ctx.enter_context(tc.tile_pool(name="feat", bufs=1))
        tpool = ctx.enter_context(tc.tile_pool(name="tmp", bufs=3))
        cpool = ctx.enter_context(tc.tile_pool(name="corr", bufs=3))
        opool = ctx.enter_context(tc.tile_pool(name="outs", bufs=4))
        pconv = ctx.enter_context(
            tc.tile_pool(name="pconv", bufs=4, space="PSUM"))
        psmall = ctx.enter_context(
            tc.tile_pool(name="psmall", bufs=2, space="PSUM"))

        eye = consts.tile([128, 128], bf16)
        make_identity(nc, eye)
        twh0 = consts.tile([128, 8, 8, 16], f32)
        twh1 = consts.tile([128, 8, 8, 16], f32)
        tww0 = consts.tile([128, 8, 8, 8], f32)
        tww1 = consts.tile([128, 8, 8, 8], f32)
        nc.sync.dma_start(twh0[:], wh0)
        nc.sync.dma_start(twh1[:], wh1)
        nc.sync.dma_start(tww0[:], ww0)
        nc.sync.dma_start(tww1[:], ww1)
        # resize matrices as lhsT tiles: [kin partitions (chunked), 256]
        trm = {}
        trm[0] = consts.tile([16, 1, 256], bf16, tag="rm0", name="rm0")
        nc.sync.dma_start(trm[0][:, 0, :], rmat0)
        trm[1] = consts.tile([64, 1, 256], bf16, tag="rm1", name="rm1")
        nc.sync.dma_start(trm[1][:, 0, :], rmat1)

        # padded conv inputs. s2: per (side, img, chunk) [18,18]; s1: (si, ch)
        # [10,10]; s0: [6,6]
        pad2 = inpool.tile([128, 2, IMG_PER_CORE, 8, 18, 18], f32)
        pad1 = inpool.tile([128, 4, 8, 10, 10], f32)
        pad0 = inpool.tile([128, 4, 8, 6, 6], f32)
        nc.vector.memset(pad2[:], 0.0)
        nc.vector.memset(pad1[:], 0.0)
        nc.vector.memset(pad0[:], 0.0)

        for side, src in ((0, qf), (1, kf)):
            for img in range(IMG_PER_CORE):
                for ch in range(8):
                    nc.sync.dma_start(
                        pad2[:, side, img, ch, 1:17, 1:17],
                        src[img, ch * 128:(ch + 1) * 128, :, :])

        # -------- bilinear resizes (DVE), overlap with s2 conv --------
        for side in range(2):
            for img in range(IMG_PER_CORE):
                si = side * IMG_PER_CORE + img
                ity = pad2[:, side, img, :, 1:17, 1:17]
                th0 = tpool.tile([128, 8, 8, 16], f32, tag="th")
                th1 = tpool.tile([128, 8, 8, 16], f32, tag="th")
                # H pass: rows 0,2,..,14 and 1,3,..,15 of the 16-row interior
                nc.vector.tensor_mul(th0[:], pad2[:, side, img, :, 1:16:2, 1:17],
                                     twh0[:])
                nc.vector.tensor_mul(th1[:], pad2[:, side, img, :, 2:17:2, 1:17],
                                     twh1[:])
                nc.vector.tensor_add(th0[:], th0[:], th1[:])
                # W pass into padded s1 interior
                tw0 = tpool.tile([128, 8, 8, 8], f32, tag="tw")
                nc.vector.tensor_mul(tw0[:], th0[:, :, :, 0:15:2], tww0[:])
                nc.vector.tensor_mul(pad1[:, si, :, 1:9, 1:9],
                                     th0[:, :, :, 1:16:2], tww1[:])
                nc.vector.tensor_add(pad1[:, si, :, 1:9, 1:9],
                                     pad1[:, si, :, 1:9, 1:9], tw0[:])
                # s0: exact subsample rows/cols 0,5,10,15
                nc.vector.tensor_copy(pad0[:, si, :, 1:5, 1:5],
                                      ity[:, :, 0:16:5, 0:16:5])

        # bf16 mirrors of the padded inputs for the PE
        pad2b = inpool.tile([128, 2, IMG_PER_CORE, 8, 18, 18], bf16)
        pad1b = inpool.tile([128, 4, 8, 10, 10], bf16)
        pad0b = inpool.tile([128, 4, 8, 6, 6], bf16)
        nc.vector.tensor_copy(pad2b[:], pad2[:])
        nc.vector.tensor_copy(pad1b[:], pad1[:])
        nc.vector.tensor_copy(pad0b[:], pad0[:])

        # -------- 3x3 convs as 9-tap matmul accumulation (bf16) --------
        # features: per scale, per co-chunk tiles; si = side*IMG+img
        feats = {
            2: [fpool.tile([128, 2, IMG_PER_CORE, 256], bf16, tag="f2_%d" % co,
                           name="f2_%d" % co) for co in range(2)],
            1: [fpool.tile([128, 4, 64], bf16, tag="f1_%d" % co,
                           name="f1_%d" % co) for co in range(2)],
            0: [fpool.tile([128, 4, 16], bf16, tag="f0_%d" % co,
                           name="f0_%d" % co) for co in range(2)],
        }

        def conv_scale(s, pad_t, hw, rhs_of_tap, psum_shape, out_copy):
            npos = hw * hw
            ps = {}
            for grp in range(len(psum_shape)):
                for co in range(2):
                    ps[(grp, co)] = pconv.tile(psum_shape[grp], f32, tag="cv", name="cv")
            for ci in range(8):
                for tap in range(9):
                    wt = wpool.tile([128, 256], bf16, tag="w")
                    nc.sync.dma_start(
                        wt[:], wconv[s, tap, ci * 128:(ci + 1) * 128, :])
                    first = (ci == 0 and tap == 0)
                    last = (ci == 7 and tap == 8)
                    for grp in range(len(psum_shape)):
                        rhs = rhs_of_tap(grp, ci, tap)
                        for co in range(2):
                            nc.tensor.matmul(
                                ps[(grp, co)][:],
                                wt[:, co * 128:(co + 1) * 128],
                                rhs,
                                start=first, stop=last)
            out_copy(ps)

        # scale 2 (16x16): one matmul group per side, N = IMG*256
        def rhs2(grp, ci, tap):
            dy, dx = tap // 3, tap % 3
            return pad2b[:, grp, :, ci, dy:dy + 16, dx:dx + 16]

        def copy2(ps):
            for side in range(2):
                for co in range(2):
                    nc.vector.tensor_copy(feats[2][co][:, side], ps[(side, co)][:])

        conv_scale(2, pad2, 16, rhs2,
                   [[128, IMG_PER_CORE, 16, 16], [128, IMG_PER_CORE, 16, 16]],
                   copy2)

        # scale 1 (8x8): all 4 (side,img) in one group, N = 4*64 = 256
        def rhs1(grp, ci, tap):
            dy, dx = tap // 3, tap % 3
            return pad1b[:, :, ci, dy:dy + 8, dx:dx + 8]

        def copy1(ps):
            for co in range(2):
                nc.vector.tensor_copy(feats[1][co][:], ps[(0, co)][:])

        conv_scale(1, pad1, 8, rhs1, [[128, 4, 8, 8]], copy1)

        # scale 0 (4x4): N = 4*16 = 64
        def rhs0(grp, ci, tap):
            dy, dx = tap // 3, tap % 3
            return pad0b[:, :, ci, dy:dy + 4, dx:dx + 4]

        def copy0(ps):
            for co in range(2):
                nc.vector.tensor_copy(feats[0][co][:], ps[(0, co)][:])

        conv_scale(0, pad0, 4, rhs0, [[128, 4, 4, 4]], copy0)

        # -------- channel L2 normalization (norm over 256 = 2 chunks) -------
        for s in range(3):
            f0, f1 = feats[s]
            nfree = int(np.prod(f0.shape[1:]))
            sq0 = tpool.tile([128, nfree], f32, tag="nrm")
            sq1 = tpool.tile([128, nfree], f32, tag="nrm")
            a0 = f0.rearrange("p ... -> p (...)")
            a1 = f1.rearrange("p ... -> p (...)")
            nc.vector.tensor_mul(sq0[:], a0, a0)
            nc.vector.tensor_mul(sq1[:], a1, a1)
            nc.vector.tensor_add(sq0[:], sq0[:], sq1[:])
            red = tpool.tile([128, nfree], f32, tag="nrm")
            nc.gpsimd.partition_all_reduce(red[:], sq0[:], 128,
                                           bass_isa.ReduceOp.add)
            nc.scalar.activation(red[:], red[:], Sqrt)
            nc.vector.tensor_scalar_add(red[:], red[:], EPS)
            nc.vector.reciprocal(red[:], red[:])
            redb = tpool.tile([128, nfree], bf16, tag="nrmb")
            nc.vector.tensor_copy(redb[:], red[:])
            nc.vector.tensor_mul(a0, a0, redb[:])
            nc.vector.tensor_mul(a1, a1, redb[:])

        # -------- correlations + 4D resize + relu --------
        def feat_ap(s, side, img, co):
            if s == 2:
                return feats[2][co][:, side, img]
            si = side * IMG_PER_CORE + img
            return feats[s][co][:, si]

        for img in range(IMG_PER_CORE):
            for qs in range(3):
                for ks in range(3):
                    pair = qs * 3 + ks
                    qpos = SCALE_HW[qs] ** 2
                    kpos = SCALE_HW[ks] ** 2
                    kch = (kpos + 127) // 128
                    qch = (qpos + 127) // 128

                    # corrT[k, q] = sum_c K[c,k] Q[c,q]
                    ct = cpool.tile([128, kch, qpos], bf16, tag="ct")
                    for kc in range(kch):
                        kp = min(128, kpos - kc * 128)
                        cps = psmall.tile([128, 256], f32, tag="ps")
                        for co in range(2):
                            nc.tensor.matmul(
                                cps[:kp, :qpos],
                                feat_ap(ks, 1, img, co)[
                                    :, kc * 128:kc * 128 + kp],
                                feat_ap(qs, 0, img, co),
                                start=(co == 0), stop=(co == 1))
                        nc.vector.tensor_copy(ct[:kp, kc, :], cps[:kp, :qpos])

                    # pass A: resize k dims (partition) to 256 unless ks == 2
                    if ks == 2:
                        outA = ct  # [128, 2, qpos]
                    else:
                        outA = cpool.tile([128, 2, qpos], bf16, tag="oa")
                        for m in range(2):
                            pa = psmall.tile([128, 256], f32, tag="ps")
                            for kc in range(kch):
                                kp = min(128, kpos - kc * 128)
                                nc.tensor.matmul(
                                    pa[:, :qpos],
                                    trm[ks][kc * 128:kc * 128 + kp, 0,
                                            m * 128:(m + 1) * 128],
                                    ct[:kp, kc, :],
                                    start=(kc == 0), stop=(kc == kch - 1))
                            nc.vector.tensor_copy(outA[:, m, :], pa[:, :qpos])

                    # transpose -> ctq [qpos part (chunks), 256 k]
                    ctq = cpool.tile([128, qch, 256], bf16, tag="ctq")
                    for m in range(2):
                        for qc in range(qch):
                            qp = min(128, qpos - qc * 128)
                            pt = psmall.tile([128, 256], bf16, tag="pst")
                            nc.tensor.transpose(
                                pt[:qp, :128],
                                outA[:, m, qc * 128:qc * 128 + qp], eye[:])
                            nc.vector.tensor_copy(
                                ctq[:qp, qc, m * 128:(m + 1) * 128],
                                pt[:qp, :128])

                    # pass B: resize q dims to 256 unless qs == 2; relu; store
                    if qs == 2:
                        for qm in range(2):
                            xs = opool.tile([128, 256], f32, tag="xs")
                            nc.vector.tensor_scalar_max(xs[:], ctq[:, qm, :], 0.0)
                            nc.sync.dma_start(
                                xout[img, pair, qm * 128:(qm + 1) * 128, :],
                                xs[:])
                    else:
                        for qm in range(2):
                            pb = psmall.tile([128, 256], f32, tag="ps")
                            for qc in range(qch):
                                qp = min(128, qpos - qc * 128)
                                nc.tensor.matmul(
                                    pb[:],
                                    trm[qs][qc * 128:qc * 128 + qp, 0,
                                            qm * 128:(qm + 1) * 128],
                                    ctq[:qp, qc, :],
                                    start=(qc == 0), stop=(qc == qch - 1))
                            xs = opool.tile([128, 256], f32, tag="xs")
                            nc.vector.tensor_scalar_max(xs[:], pb[:], 0.0)
                            nc.sync.dma_start(
                                xout[img, pair, qm * 128:(qm + 1) * 128, :],
                                xs[:])

    nc.compile()
    return nc


# ----------------------------------------------------------------------------
# host tail: CVLearner + MLP (numpy)
# ----------------------------------------------------------------------------

def _conv2d_np(x, w, stride, pad):
    n, ci, hh, ww = x.shape
    o, _, kh, kw = w.shape
    xp = np.pad(x, ((0, 0), (0, 0), (pad, pad), (pad, pad)))
    oh = (hh + 2 * pad - kh) // stride + 1
    ow = (ww + 2 * pad - kw) // stride + 1
    cols = []
    for dy in range(kh):
        for dx in range(kw):
            cols.append(xp[:, :, dy:dy + oh * stride:stride,
                           dx:dx + ow * stride:stride])
    cols = np.concatenate(cols, axis=1).reshape(n, kh * kw * ci, oh * ow)
    wm = w.transpose(0, 2, 3, 1).reshape(o, kh * kw * ci)
    out = np.einsum('oc,ncp->nop', wm, cols, optimize=True)
    return out.reshape(n, o, oh, ow)


def _cp_conv4d(x, w1, b1, w2, b2, stride, pad):
    Bn, Cc, ha, wa, hb, wb = x.shape
    co = w1.shape[0]
    x1 = x[..., ::stride, ::stride]
    hbs, wbs = x1.shape[-2], x1.shape[-1]
    x1 = np.transpose(x1, (0, 4, 5, 1, 2, 3)).reshape(Bn * hbs * wbs, Cc, ha, wa)
    o1 = _conv2d_np(x1, w1, stride, pad) + b1[None, :, None, None]
    oha, owa = o1.shape[-2], o1.shape[-1]
    o1 = np.transpose(o1.reshape(Bn, hbs, wbs, co, oha, owa), (0, 3, 4, 5, 1, 2))
    x2 = x[:, :, ::stride, ::stride]
    has, was = x2.shape[2], x2.shape[3]
    x2 = np.transpose(x2, (0, 2, 3, 1, 4, 5)).reshape(Bn * has * was, Cc, hb, wb)
    o2 = _conv2d_np(x2, w2, stride, pad) + b2[None, :, None, None]
    ohb, owb = o2.shape[-2], o2.shape[-1]
    o2 = np.transpose(o2.reshape(Bn, has, was, co, ohb, owb), (0, 3, 1, 2, 4, 5))
    return o1 + o2


def _group_norm(x, gamma, beta, groups=4, eps=1e-5):
    Bn, Cc = x.shape[0], x.shape[1]
    xg = x.reshape(Bn, groups, -1)
    mu = xg.mean(axis=-1, keepdims=True)
    var = xg.var(axis=-1, keepdims=True)
    xg = (xg - mu) / np.sqrt(var + eps)
    x = xg.reshape(x.shape)
    bshape = (1, Cc) + (1,) * (x.ndim - 2)
    return x * gamma.reshape(bshape) + beta.reshape(bshape)


def _host_tail(x, p):
    # x: [B, 9, 16,16,16,16] relu'd resized correlations
    for li, (_, _, ksz, stride) in enumerate(LAYERS):
        x = _cp_conv4d(x, p['w1_%d' % li], p['b1_%d' % li],
                       p['w2_%d' % li], p['b2_%d' % li], stride, ksz // 2)
        x = np.maximum(_group_norm(x, p['gn_g_%d' % li], p['gn_b_%d' % li]), 0.0)
    feat = x.mean(axis=(2, 3, 4, 5))
    h = np.maximum(feat @ p['mlp_w1'].T + p['mlp_b1'], 0.0)
    logits = h @ p['mlp_w2'].T + p['mlp_b2']
    z = logits - logits.max(axis=1, keepdims=True)
    e = np.exp(z)
    return (e / e.sum(axis=1, keepdims=True))[:, 1].astype(np.float32)


# ----------------------------------------------------------------------------
# entry point
# ----------------------------------------------------------------------------

_LAST_RESULTS = None


def kernel(query_features, key_features, params):
    global _LAST_RESULTS
    from concourse.bass_utils import run_bass_kernel_spmd

    qf = np.ascontiguousarray(np.asarray(query_features, np.float32))
    kf = np.ascontiguousarray(np.asarray(key_features, np.float32))
    p = {k: np.asarray(v, np.float32) for k, v in params.items()}

    consts = _build_consts(p)
    nc = _build_program()

    in_maps = []
    for core in range(N_CORES):
        lo = core * IMG_PER_CORE
        m = dict(consts)
        m['qf'] = np.ascontiguousarray(qf[lo:lo + IMG_PER_CORE])
        m['kf'] = np.ascontiguousarray(kf[lo:lo + IMG_PER_CORE])
        in_maps.append(m)

    res = run_bass_kernel_spmd(nc, in_maps, list(range(N_CORES)))
    _LAST_RESULTS = res
    x = np.concatenate([r['xout'] for r in res.results], axis=0)
    x = x.reshape(B, 9, 16, 16, 16, 16)
    return _host_tail(x, p)
